# revision 1
# baseline (speedup 1.0000x reference)
"""Gaussian kernel matrix (pairwise L2 over T) for x:(32,64,1000,16) -> (32,64,64,16).

out[n,c,d,f] = exp(-||x[n,c,:,f] - x[n,d,:,f]||^2 / 2)

Strategy (8 NeuronCores, data-parallel over N, 4 batch elems per core):
  Per core, per pair of batch elems (2n x 64c = 128 partitions):
    1. DMA natural-layout slab HBM->SBUF with fp32->bf16 cast (contiguous reads).
    2. PE-transpose [128(2n,c), 128t] tiles -> [128t, 128(2n,c)] (per f, per t-chunk),
       staged through PSUM, copied to SBUF (split DVE/ACT).
    3. Gram matmuls: G_f = X_f^T X_f accumulated over 8 t-chunks of 128 (T padded
       to 1024 with zeros). One [K=128,M=128,N=128] matmul per (f, chunk) computes
       both batch elems' grams (diagonal 64x64 blocks; cross-n blocks unused).
    4. Epilogue: sq_c = diag(G) via identity-mask + row reduce;
       H = exp((G - sq_c)/2); O = H * H^T_block  (= exp(G - sq_c/2 - sq_d/2)).
       Diagonal is exactly 1 (exact cancellation); H^T via small PE transposes.
    5. DMA out fp32 in (n,c,d,f) layout (strided DVE write fixes f-innermost).
bf16 matmul inputs with fp32 PSUM accumulation; the epilogue's exact diagonal
cancellation makes the output independent of the bf16 rounding on-diagonal.
"""

import numpy as np

N_FULL, C, T, F = 32, 64, 1000, 16
N_CORES = 8
N_PER_CORE = N_FULL // N_CORES  # 4
NPAIRS = N_PER_CORE // 2        # 2
TPAD = 1024
TCH = TPAD // 128               # 8 t-chunks
FG = 2                          # f-groups
F_PER_G = F // FG               # 8

_CACHE = {}


def _split_multi_waits(bir_bytes):
    """Walrus codegen here only supports one sync-wait per instruction; Tile
    emits several. Split extras into preceding NoOp instructions on the same
    engine queue (engine executes in order, so the waits still gate)."""
    import json

    bir = json.loads(bir_bytes)
    cnt = 0
    for fn in bir["functions"]:
        for blk in fn["blocks"]:
            new = []
            for inst in blk["instructions"]:
                si = inst.get("sync_info")
                waits = (si or {}).get("on_wait", [])
                if len(waits) > 1:
                    for w in waits[:-1]:
                        cnt += 1
                        new.append(
                            {
                                "debug": inst.get("debug", 0),
                                "engine": inst["engine"],
                                "ins": [],
                                "outs": [],
                                "name": f"WS{cnt}",
                                "opcode": "NoOp",
                                "sync_info": {"on_update": [], "on_wait": [w]},
                            }
                        )
                    si["on_wait"] = waits[-1:]
                new.append(inst)
            blk["instructions"] = new
    return json.dumps(bir).encode()


def _build_nc():
    import concourse.bass as bass
    import concourse.mybir as mybir
    import concourse.tile as tile
    from concourse.masks import make_identity

    dt = mybir.dt
    nc = bass.Bass()
    x = nc.dram_tensor("x", (N_PER_CORE, C, T, F), dt.float32, kind="ExternalInput")
    y = nc.dram_tensor("y", (N_PER_CORE, C, C, F), dt.float32, kind="ExternalOutput")

    with tile.TileContext(nc) as tc:
        with (
            tc.tile_pool(name="const", bufs=1) as constp,
            tc.tile_pool(name="slab", bufs=2) as slabp,
            tc.tile_pool(name="trT", bufs=1) as trp,
            tc.tile_pool(name="work", bufs=2) as workp,
            tc.tile_pool(name="osb", bufs=2) as outp,
            tc.tile_pool(name="ps_tr", bufs=1, space="PSUM") as ps_tr,
            tc.tile_pool(name="ps_gram", bufs=2, space="PSUM") as ps_gram,
            tc.tile_pool(name="ps_tt", bufs=2, space="PSUM") as ps_tt,
        ):
            ident_bf = constp.tile([128, 128], dt.bfloat16)
            ident_f32 = constp.tile([128, 128], dt.float32)
            make_identity(nc, ident_bf)
            make_identity(nc, ident_f32)

            for p in range(NPAIRS):
                slab = slabp.tile([128, TPAD, F], dt.bfloat16, tag="slab")
                nc.gpsimd.memset(slab[:, T:, :], 0.0)
                src = x[2 * p : 2 * p + 2].rearrange("n c t f -> (n c) t f")
                nc.gpsimd.dma_start(slab[:, :T, :], src)  # fp32 -> bf16 cast

                trT = trp.tile([128, TCH, F, 128], dt.bfloat16, tag="trT")
                for ch in range(TCH):
                    ps = ps_tr.tile([128, F, 128], dt.bfloat16, tag="pstr")
                    for f in range(F):
                        nc.tensor.transpose(
                            ps[:, f, :],
                            slab[:, ch * 128 : (ch + 1) * 128, f],
                            ident_bf,
                        )
                    nc.vector.tensor_copy(trT[:, ch, 0:8, :], ps[:, 0:8, :])
                    nc.scalar.copy(trT[:, ch, 8:16, :], ps[:, 8:16, :])

                out_sb = outp.tile([128, C, F], dt.float32, tag="osb")
                for g in range(FG):
                    gram = ps_gram.tile([128, F_PER_G, 128], dt.float32, tag="gram")
                    for f8 in range(F_PER_G):
                        f = g * F_PER_G + f8
                        for ch in range(TCH):
                            nc.tensor.matmul(
                                gram[:, f8, :],
                                trT[:, ch, f, :],
                                trT[:, ch, f, :],
                                start=(ch == 0),
                                stop=(ch == TCH - 1),
                                skip_group_check=True,
                            )
                    masked = workp.tile([128, F_PER_G, 64], dt.float32, tag="masked")
                    sq = workp.tile([128, F_PER_G], dt.float32, tag="sq")
                    dti = workp.tile([128, F_PER_G, 64], dt.float32, tag="dti")
                    h = workp.tile([128, F_PER_G, 64], dt.bfloat16, tag="h")
                    for m in range(2):
                        sl = slice(64 * m, 64 * m + 64)
                        Gm = gram[sl, :, sl]  # [64, 8, 64] diag block
                        nc.vector.tensor_tensor(
                            masked[sl],
                            Gm,
                            ident_f32[sl, sl][:, None, :].to_broadcast((64, F_PER_G, 64)),
                            mybir.AluOpType.mult,
                        )
                        nc.vector.reduce_sum(
                            sq[sl], masked[sl], axis=mybir.AxisListType.X
                        )
                        nc.vector.tensor_tensor(
                            dti[sl],
                            Gm,
                            sq[sl][:, :, None].to_broadcast((64, F_PER_G, 64)),
                            mybir.AluOpType.subtract,
                        )
                        nc.scalar.activation(
                            h[sl], dti[sl], mybir.ActivationFunctionType.Exp, scale=0.5
                        )
                    tt = ps_tt.tile([128, F_PER_G, 64], dt.bfloat16, tag="tt")
                    for m in range(2):
                        sl = slice(64 * m, 64 * m + 64)
                        for f8 in range(F_PER_G):
                            nc.tensor.transpose(
                                tt[sl, f8, :], h[sl, f8, :], ident_bf[sl, sl]
                            )
                    nc.vector.tensor_tensor(
                        out_sb[:, :, g * F_PER_G : (g + 1) * F_PER_G].rearrange(
                            "p d f -> p f d"
                        ),
                        h,
                        tt,
                        mybir.AluOpType.mult,
                    )
                dst = y[2 * p : 2 * p + 2].rearrange("n c d f -> (n c) d f")
                nc.sync.dma_start(dst, out_sb)

    orig_ser = nc.to_json_bytes
    nc.to_json_bytes = lambda: _split_multi_waits(orig_ser())
    return nc


def _get_nc():
    if "nc" not in _CACHE:
        _CACHE["nc"] = _build_nc()
    return _CACHE["nc"]


def kernel(x, _trace=False):
    from concourse.bass_utils import run_bass_kernel_spmd

    x = np.ascontiguousarray(np.asarray(x), dtype=np.float32)
    assert x.shape == (N_FULL, C, T, F), x.shape
    nc = _get_nc()
    in_maps = [
        {"x": np.ascontiguousarray(x[N_PER_CORE * i : N_PER_CORE * (i + 1)])}
        for i in range(N_CORES)
    ]
    res = run_bass_kernel_spmd(nc, in_maps, core_ids=list(range(N_CORES)), trace=_trace)
    out = np.concatenate([r["y"] for r in res.results], axis=0)
    if _trace:
        _CACHE["last_result"] = res
    return out



# revision 3
# speedup vs baseline: 1.3269x; 1.3269x over previous
"""Gaussian kernel matrix (pairwise L2 over T) for x:(32,64,1000,16) -> (32,64,64,16).

out[n,c,d,f] = exp(-||x[n,c,:,f] - x[n,d,:,f]||^2 / 2)

Strategy (8 NeuronCores, data-parallel over N, 4 batch elems per core):
  Per core, per pair of batch elems (2n x 64c = 128 partitions):
    1. DMA natural-layout slab HBM->SBUF with fp32->bf16 cast, split into
       t-quarters so compute starts after the first quarter lands.
    2. Per t-chunk of 128: transpose [128(2n,c), 128t] tiles per f via
       *normal* matmuls (chunk as stationary, identity as moving operand,
       fp32 PSUM) -- unlike is_transpose matmuls these count as PE-busy so
       the HAM clock gate warms to 2.4 GHz. Transposes run in f-quads into
       1-bank PSUM tiles (double-buffered); copies to SBUF alternate DVE/ACT.
    3. Gram matmuls interleaved per chunk: G_f += chunkT_f^T chunkT_f
       accumulated into a [128,F,128] fp32 PSUM block across the 8 chunks.
       PSUM has_written bits are bank-wide on start: only the first matmul
       into each bank (ch==0, f%4==0) sets start=True; the other f's rely
       on cleared bits -> overwrite semantics.
    4. Epilogue: sq_c = diag(G) via identity-mask + row reduce;
       H = exp((G - sq_c)/2); O = H * H^T_block (= exp(G - sq_c/2 - sq_d/2)).
       Diagonal is exactly 1 (exact cancellation); H^T via small normal
       matmuls (tile-positioned 64x64).
    5. DMA out fp32 in (n,c,d,f) layout (strided DVE write fixes f-innermost).
Pair boundaries are software-pipelined: pair p+1's first chunk of transposes
is emitted before pair p's epilogue so the PE never waits on the DVE chain.
bf16 matmul inputs with fp32 PSUM accumulation; the epilogue's exact diagonal
cancellation makes the output independent of the bf16 rounding on-diagonal.
"""

import numpy as np

N_FULL, C, T, F = 32, 64, 1000, 16
N_CORES = 8
N_PER_CORE = N_FULL // N_CORES  # 4
NPAIRS = N_PER_CORE // 2        # 2
TPAD = 1024
TCH = TPAD // 128               # 8 t-chunks
FG = 2                          # f-groups (epilogue)
F_PER_G = F // FG               # 8
FQ = 4                          # f-quad (transpose PSUM granularity)
NQ = 4                          # input sub-DMAs per pair (250 t's each)
TQ = T // NQ

_CACHE = {}


def _split_multi_waits(bir_bytes):
    """Walrus codegen here only supports one sync-wait per instruction; Tile
    emits several. Split extras into preceding NoOp instructions on the same
    engine queue (engine executes in order, so the waits still gate)."""
    import json

    bir = json.loads(bir_bytes)
    cnt = 0
    for fn in bir["functions"]:
        for blk in fn["blocks"]:
            new = []
            for inst in blk["instructions"]:
                si = inst.get("sync_info")
                waits = (si or {}).get("on_wait", [])
                if len(waits) > 1:
                    for w in waits[:-1]:
                        cnt += 1
                        new.append(
                            {
                                "debug": inst.get("debug", 0),
                                "engine": inst["engine"],
                                "ins": [],
                                "outs": [],
                                "name": f"WS{cnt}",
                                "opcode": "NoOp",
                                "sync_info": {"on_update": [], "on_wait": [w]},
                            }
                        )
                    si["on_wait"] = waits[-1:]
                new.append(inst)
            blk["instructions"] = new
    return json.dumps(bir).encode()


def _build_nc():
    import concourse.bass as bass
    import concourse.mybir as mybir
    import concourse.tile as tile
    from concourse.masks import make_identity

    dt = mybir.dt
    nc = bass.Bass()
    x = nc.dram_tensor("x", (N_PER_CORE, C, T, F), dt.float32, kind="ExternalInput")
    y = nc.dram_tensor("y", (N_PER_CORE, C, C, F), dt.float32, kind="ExternalOutput")

    with tile.TileContext(nc) as tc:
        with (
            tc.tile_pool(name="const", bufs=1) as constp,
            tc.tile_pool(name="slab", bufs=2) as slabp,
            tc.tile_pool(name="trT", bufs=3) as trp,
            tc.tile_pool(name="work", bufs=2) as workp,
            tc.tile_pool(name="osb", bufs=2) as outp,
            tc.tile_pool(name="ps_tr", bufs=2, space="PSUM") as ps_tr,
            tc.tile_pool(name="ps_gram", bufs=1, space="PSUM") as ps_gram,
            tc.tile_pool(name="ps_tt", bufs=2, space="PSUM") as ps_tt,
        ):
            ident_bf = constp.tile([128, 128], dt.bfloat16)
            ident_f32 = constp.tile([128, 128], dt.float32)
            make_identity(nc, ident_bf)
            make_identity(nc, ident_f32)

            def emit_dma(p):
                slab = slabp.tile([128, TPAD, F], dt.bfloat16, tag="slab")
                nc.gpsimd.memset(slab[:, T:, :], 0.0)
                src = x[2 * p : 2 * p + 2].rearrange("n c t f -> (n c) t f")
                for q in range(NQ):
                    nc.gpsimd.dma_start(
                        slab[:, q * TQ : (q + 1) * TQ, :],
                        src[:, q * TQ : (q + 1) * TQ, :],
                    )  # fp32 -> bf16 cast
                return slab

            def emit_T(slab, ch):
                """Transpose one t-chunk (all 16 f) via normal matmuls."""
                trT_ch = trp.tile([128, F, 128], dt.bfloat16, tag="trT")
                for q in range(F // FQ):
                    trps = ps_tr.tile([128, FQ, 128], dt.float32, tag="trps")
                    for j in range(FQ):
                        f = q * FQ + j
                        nc.tensor.matmul(
                            trps[:, j, :],
                            slab[:, ch * 128 : (ch + 1) * 128, f],
                            ident_bf,
                        )
                    dstq = trT_ch[:, q * FQ : (q + 1) * FQ, :]
                    if q % 2 == 0:
                        nc.vector.tensor_copy(dstq, trps)
                    else:
                        nc.scalar.copy(dstq, trps)
                return trT_ch

            def emit_G(gram, trT_ch, ch):
                for f in range(F):
                    nc.tensor.matmul(
                        gram[:, f, :],
                        trT_ch[:, f, :],
                        trT_ch[:, f, :],
                        start=(ch == 0 and f % 4 == 0),
                        stop=(ch == TCH - 1),
                        skip_group_check=True,
                    )

            def emit_E(p, gram):
                out_sb = outp.tile([128, C, F], dt.float32, tag="osb")
                hs = []
                for g in range(FG):
                    fsl = slice(g * F_PER_G, (g + 1) * F_PER_G)
                    masked = workp.tile([128, F_PER_G, 64], dt.float32, tag="masked")
                    sq = workp.tile([128, F_PER_G], dt.float32, tag="sq")
                    dti = workp.tile([128, F_PER_G, 64], dt.float32, tag="dti")
                    h = workp.tile([128, F_PER_G, 64], dt.bfloat16, tag="h")
                    for m in range(2):
                        sl = slice(64 * m, 64 * m + 64)
                        Gm = gram[sl, fsl, sl]  # [64, 8, 64] diag block
                        nc.vector.tensor_tensor(
                            masked[sl],
                            Gm,
                            ident_f32[sl, sl][:, None, :].to_broadcast(
                                (64, F_PER_G, 64)
                            ),
                            mybir.AluOpType.mult,
                        )
                        nc.vector.reduce_sum(
                            sq[sl], masked[sl], axis=mybir.AxisListType.X
                        )
                        nc.vector.tensor_tensor(
                            dti[sl],
                            Gm,
                            sq[sl][:, :, None].to_broadcast((64, F_PER_G, 64)),
                            mybir.AluOpType.subtract,
                        )
                        nc.scalar.activation(
                            h[sl], dti[sl], mybir.ActivationFunctionType.Exp, scale=0.5
                        )
                    hs.append(h)
                for g in range(FG):
                    h = hs[g]
                    tt = ps_tt.tile([128, F_PER_G, 64], dt.float32, tag="tt")
                    for m in range(2):
                        sl = slice(64 * m, 64 * m + 64)
                        for f8 in range(F_PER_G):
                            nc.tensor.matmul(
                                tt[sl, f8, :], h[sl, f8, :], ident_bf[sl, sl]
                            )
                    nc.vector.tensor_tensor(
                        out_sb[:, :, g * F_PER_G : (g + 1) * F_PER_G].rearrange(
                            "p d f -> p f d"
                        ),
                        h,
                        tt,
                        mybir.AluOpType.mult,
                    )
                dst = y[2 * p : 2 * p + 2].rearrange("n c d f -> (n c) d f")
                nc.sync.dma_start(dst, out_sb)

            slabs = [emit_dma(p) for p in range(NPAIRS)]

            # pair 0: full transpose+gram pipeline
            gram0 = ps_gram.tile([128, F, 128], dt.float32, tag="gram")
            for ch in range(TCH):
                trT_ch = emit_T(slabs[0], ch)
                emit_G(gram0, trT_ch, ch)
            # pair 1's first chunk of transposes covers pair 0's epilogue
            trT_10 = emit_T(slabs[1], 0)
            emit_E(0, gram0)
            gram1 = ps_gram.tile([128, F, 128], dt.float32, tag="gram")
            emit_G(gram1, trT_10, 0)
            for ch in range(1, TCH):
                trT_ch = emit_T(slabs[1], ch)
                emit_G(gram1, trT_ch, ch)
            emit_E(1, gram1)

    orig_ser = nc.to_json_bytes
    nc.to_json_bytes = lambda: _split_multi_waits(orig_ser())
    return nc


def _get_nc():
    if "nc" not in _CACHE:
        _CACHE["nc"] = _build_nc()
    return _CACHE["nc"]


def kernel(x, _trace=False):
    from concourse.bass_utils import run_bass_kernel_spmd

    x = np.ascontiguousarray(np.asarray(x), dtype=np.float32)
    assert x.shape == (N_FULL, C, T, F), x.shape
    nc = _get_nc()
    in_maps = [
        {"x": np.ascontiguousarray(x[N_PER_CORE * i : N_PER_CORE * (i + 1)])}
        for i in range(N_CORES)
    ]
    res = run_bass_kernel_spmd(nc, in_maps, core_ids=list(range(N_CORES)), trace=_trace)
    out = np.concatenate([r["y"] for r in res.results], axis=0)
    if _trace:
        _CACHE["last_result"] = res
    return out
